# revision 8
# baseline (speedup 1.0000x reference)
"""Bahdanau-style attention scores kernel for Trainium2 (8 NeuronCores).

Reference computation (B=32, S=2048, ENC_H=512, DEC_H=1024):
    W_s = attn_w[:, :1024]; W_e = attn_w[:, 1024:]
    proj_s = s @ W_s.T                      # [B, 1024]
    proj_e = enc @ W_e.T                    # [B, S, 1024]
    scores = tanh(proj_s[:, None] + proj_e) @ v_w.T   # [B, S]
    out = softmax(scores, axis=1)

Strategy: data-parallel over batch (4 batches per core). Everything is
core-local, including the softmax, so there are no collectives.

On-device layout: all matmuls keep the hidden dim h on PSUM partitions:
    projT[h, s] = sum_e W_eT[e, h] * encT[e, s]
so the per-batch proj_s bias is a per-partition scalar (fused into the
ACT tanh) and the v-dot runs as 2 rounds of 4 concurrent col-tiled
matmuls (M=32 with v replicated across 32 columns so every PSUM
partition is written), followed by a 1/32-weighted reduce matmul.

Prologue engineering: a short stream of dummy matmuls keeps the PE HAM
activity monitor busy from ~7us so real matmuls run at 2.4 GHz instead
of the cold 1.2 GHz; weights arrive in h-quarter chunks (hc-major) so
the first matmul/tanh groups gate on ~512KB instead of 2MB; the first
batch's encoder stream lands in small leading pieces. proj_s matmuls
are interleaved with the first piece's main groups so neither blocks
the other. The host passes encoder_outputs pre-transposed to [b, E, S]
(pure layout change, f32); f32 -> bf16 conversion of the enc stream
happens inside the SWDGE DMA (cast-on-load). The small replicated
weights are pre-cast to bf16 on the host and loaded via HWDGE.
"""

import numpy as np
import ml_dtypes

import concourse.bass as bass
import concourse.tile as tile
from concourse import mybir
from concourse.bass_utils import run_bass_kernel_spmd

N_CORES = 8
B, S = 32, 2048
E = 1024  # 2*ENC_H, contraction dim of the big matmul
H = 1024  # DEC_H, hidden dim of tanh
D = 1024  # DEC_H, contraction dim of proj_s
BPC = B // N_CORES  # batches per core
P = 128
EC, HC, DC = E // P, H // P, D // P

# s-piece schedule: batch 0 starts small so the first matmul group gates
# on ~0.5MB of enc DMA, later batches use full 512 pieces.
PIECES_B0 = [256, 256, 512, 512, 512]
PIECES = [512] * 4
N_DUMMY = 300  # HAM warm-up matmuls (N=8 each, ~30ns apiece)

F32 = mybir.dt.float32
BF16 = mybir.dt.bfloat16
NP_BF16 = ml_dtypes.bfloat16

_cache = {}


def _split_multiwaits(nc):
    """Walrus in this toolchain rejects instructions carrying more than one
    semaphore wait ("Too many sync wait commands"). Engine queues dispatch in
    order, so moving the extra waits onto same-engine NoOps just before the
    instruction is semantically identical."""
    for fn in nc.m.functions:
        for blk in fn.blocks:
            out = []
            for inst in blk.instructions:
                si = inst.sync_info
                waits = list(si.on_wait) if si is not None and si.on_wait else []
                if len(waits) > 1:
                    for i, w in enumerate(waits[:-1]):
                        out.append(
                            mybir.InstNoOp(
                                name=f"{inst.name}-w{i}",
                                engine=inst.engine,
                                sync_info=mybir.SyncInfo(on_wait=[w], on_update=[]),
                                bass_nofuse=True,
                            )
                        )
                    si.on_wait = [waits[-1]]
                    inst.sync_info = si
                out.append(inst)
            try:
                blk.instructions = out
            except Exception:
                blk.set_instructions(out)


def _dedup_ldweights(nc):
    """Tile lowers every matmul to an Ldweights/Matmult pair. When consecutive
    matmuls use the same stationary weights (the dummy warm-up stream), the
    second Ldweights reloads identical array state — drop it and carry its
    waits over to the next PE instruction (split later by _split_multiwaits)."""
    ndrop = 0
    for fn in nc.m.functions:
        for blk in fn.blocks:
            out = []
            loaded = None
            pending_waits = []
            for inst in blk.instructions:
                if getattr(inst, "engine", None) != mybir.EngineType.PE:
                    out.append(inst)
                    continue
                if pending_waits:
                    si = inst.sync_info or mybir.SyncInfo(on_wait=[], on_update=[])
                    si.on_wait = list(si.on_wait) + pending_waits
                    inst.sync_info = si
                    pending_waits = []
                if isinstance(inst, mybir.InstLdweights):
                    ap = inst.ins[0]
                    key = (
                        ap.memref,
                        ap.offset,
                        str(ap.ap),
                        str(ap.dtype),
                        str(getattr(inst, "tile_position", None)),
                    )
                    if key == loaded:
                        si = inst.sync_info
                        if si is not None and si.on_wait:
                            pending_waits = list(si.on_wait)
                        if si is not None and si.on_update:
                            # keep the instruction if someone depends on it
                            out.append(inst)
                            continue
                        ndrop += 1
                        continue
                    loaded = key
                elif isinstance(inst, mybir.InstMatmult):
                    pass  # matmuls stream against loaded weights
                else:
                    loaded = None  # unknown PE instruction: be conservative
                out.append(inst)
            assert not pending_waits
            try:
                blk.instructions = out
            except Exception:
                blk.set_instructions(out)
    return ndrop


def _pieces(b):
    sched = PIECES_B0 if b == 0 else PIECES
    off = 0
    out = []
    for w in sched:
        out.append((off, w))
        off += w
    return out


def _build_bass():
    nc = bass.Bass()
    enc_t = nc.dram_tensor("enc_t", [BPC, E, S], BF16, kind="ExternalInput")
    w_et = nc.dram_tensor("w_et", [E, H], BF16, kind="ExternalInput")
    w_st = nc.dram_tensor("w_st", [D, H], BF16, kind="ExternalInput")
    s_t = nc.dram_tensor("s_t", [D, BPC], BF16, kind="ExternalInput")
    v_rep = nc.dram_tensor("v_rep", [H, 32], BF16, kind="ExternalInput")
    red_t = nc.dram_tensor("red_t", [P, 1], BF16, kind="ExternalInput")
    out = nc.dram_tensor("out", [BPC, S], F32, kind="ExternalOutput")

    Tanh = mybir.ActivationFunctionType.Tanh
    Exp = mybir.ActivationFunctionType.Exp

    with tile.TileContext(nc) as tc:
        with (
            tc.tile_pool(name="consts", bufs=1) as consts,
            tc.tile_pool(name="enc", bufs=3) as enc_pool,
            tc.tile_pool(name="tanh", bufs=10) as tanh_pool,
            tc.tile_pool(name="scc", bufs=2) as scc_pool,
            tc.tile_pool(name="rows", bufs=2) as row_pool,
            tc.tile_pool(name="mmps", bufs=4, space="PSUM") as mm_psum,
            tc.tile_pool(name="scps", bufs=2, space="PSUM") as sc_psum,
            tc.tile_pool(name="psps", bufs=2, space="PSUM") as ps_psum,
        ):
            # HAM warm-up: PE busy from the end of the framework preamble so
            # the clock gate opens (1.2 -> 2.4 GHz) before real work arrives.
            # The dummy tile is memset (not DMA'd) so nothing gates it.
            dummy = consts.tile([P, 8], BF16)
            nc.vector.memset(dummy, 0.0)
            dps = ps_psum.tile([1, 8], F32, tag="psps")
            for _ in range(N_DUMMY):
                nc.tensor.matmul(dps, dummy[:, 0:1], dummy, start=True, stop=True)

            # Weights in h-quarter chunks, hc-major, so the first tanh/main
            # groups gate on 512KB not 2MB. Two HWDGE rings: sync carries
            # W_e, scalar carries W_s plus the tiny tensors (first).
            w_sb = consts.tile([P, EC, H], BF16)
            w_view = w_et[:].rearrange("(ec p) h -> p ec h", p=P)
            ws_sb = consts.tile([P, DC, H], BF16)
            ws_view = w_st[:].rearrange("(dc p) h -> p dc h", p=P)
            s_sb = consts.tile([P, DC, BPC], BF16)
            nc.scalar.dma_start(
                out=s_sb[:], in_=s_t[:].rearrange("(dc p) b -> p dc b", p=P)
            )
            v_sb = consts.tile([P, HC, 32], BF16)
            nc.scalar.dma_start(
                out=v_sb[:], in_=v_rep[:].rearrange("(hc p) r -> p hc r", p=P)
            )
            red_sb = consts.tile([P, 1], BF16)
            nc.scalar.dma_start(out=red_sb[:], in_=red_t[:])
            for q in range(4):
                hs = slice(q * 256, (q + 1) * 256)
                nc.sync.dma_start(out=w_sb[:, :, hs], in_=w_view[:, :, hs])
                nc.scalar.dma_start(out=ws_sb[:, :, hs], in_=ws_view[:, :, hs])

            projs_sb = consts.tile([P, HC, BPC], F32)

            def projs_chunk(hc):
                # projsT[h, b] = sum_d W_sT[d, h] * sT[d, b] for one h-chunk
                pp = ps_psum.tile([P, BPC], F32, tag="psps")
                for dc in range(DC):
                    nc.tensor.matmul(
                        pp,
                        ws_sb[:, dc, hc * P : (hc + 1) * P],
                        s_sb[:, dc, :],
                        start=(dc == 0),
                        stop=(dc == DC - 1),
                    )
                nc.vector.tensor_copy(projs_sb[:, hc, :], pp)

            # Flat pipeline over all (batch, piece) jobs. Per piece p the PE
            # stream is: [mm groups hc=0..7 of p] ... with p's v-rounds and
            # the reduce-mm of p-1 emitted after the FIRST mm group of p+1,
            # so the v-rounds never wait on p's last tanh and the reduce-mm
            # never waits on the DVE psum->sbuf copy.
            enc_tiles = {}
            for b in range(BPC):
                encT = enc_pool.tile([P, EC, S], BF16, tag="enc", name=f"encT{b}")
                enc_view = enc_t[b].rearrange("(ec p) s -> p ec s", p=P)
                if b == 0:
                    for off, w in _pieces(b):
                        sl = slice(off, off + w)
                        nc.gpsimd.dma_start(
                            out=encT[:, :, sl], in_=enc_view[:, :, sl]
                        )
                else:
                    for half in range(2):
                        sl = slice(half * 1024, (half + 1) * 1024)
                        nc.gpsimd.dma_start(
                            out=encT[:, :, sl], in_=enc_view[:, :, sl]
                        )
                enc_tiles[b] = encT

            rows = {}  # b -> (exp_row, sums)
            jobs = []
            for b in range(BPC):
                for pi, (off, w) in enumerate(_pieces(b)):
                    jobs.append((b, pi, off, w))

            state = {"v": None, "red": None}

            def emit_reduce():
                # reduce-mm + exp of the piece whose psum->sbuf copy is done
                if state["red"] is None:
                    return
                b, pi, off, w, scc = state["red"]
                state["red"] = None
                exp_row, sums = rows[b]
                sc2 = ps_psum.tile([1, 512], F32, tag="psps")
                nc.tensor.matmul(
                    sc2[:, :w], red_sb, scc[:, :w], start=True, stop=True
                )
                nc.scalar.activation(
                    exp_row[:, off : off + w],
                    sc2[:, :w],
                    Exp,
                    accum_out=sums[:, pi : pi + 1],
                )
                if pi == len(_pieces(b)) - 1:
                    npc = len(_pieces(b))
                    tot = row_pool.tile([1, 1], F32, tag="tot")
                    nc.vector.reduce_sum(
                        tot, sums[:, :npc], axis=mybir.AxisListType.X
                    )
                    rtot = row_pool.tile([1, 1], F32, tag="rtot")
                    nc.vector.reciprocal(rtot, tot)
                    out_row = row_pool.tile([1, S], F32, tag="out_row")
                    nc.vector.tensor_scalar_mul(out_row, exp_row, rtot)
                    nc.sync.dma_start(out=out[b : b + 1, :], in_=out_row[:])

            def emit_v():
                # v-dot of the piece whose tanh tiles are all complete:
                # 2 rounds x 4 concurrent col-tiled matmuls. v is replicated
                # over 32 columns so all 128 PSUM partitions hold valid
                # partials (no garbage for the reduce matmul).
                if state["v"] is None:
                    return
                b, pi, off, w, ths = state["v"]
                state["v"] = None
                sc_ps = sc_psum.tile([P, 512], F32, tag="scps")
                for r in range(2):
                    for j in range(4):
                        hc = r * 4 + j
                        nc.tensor.matmul(
                            sc_ps[32 * j : 32 * (j + 1), :w],
                            v_sb[:, hc, :],
                            ths[hc][:, :w],
                            start=(r == 0),
                            stop=(r == 1),
                            tile_position=(0, 32 * j),
                        )
                emit_reduce()
                scc = scc_pool.tile([P, 512], BF16, tag="scc")
                nc.vector.tensor_copy(scc[:, :w], sc_ps[:, :w])
                state["red"] = (b, pi, off, w, scc)

            for b, pi, off, w in jobs:
                if pi == 0:
                    exp_row = row_pool.tile([1, S], F32, tag="exp_row")
                    sums = row_pool.tile(
                        [1, len(PIECES_B0)], F32, tag="sums"
                    )
                    rows[b] = (exp_row, sums)
                encT = enc_tiles[b]
                sl = slice(off, off + w)
                ths = []
                for hc in range(HC):
                    if b == 0 and pi == 0:
                        projs_chunk(hc)
                    mm_ps = mm_psum.tile([P, 512], F32, tag="mmps")
                    for ec in range(EC):
                        nc.tensor.matmul(
                            mm_ps[:, :w],
                            w_sb[:, ec, hc * P : (hc + 1) * P],
                            encT[:, ec, sl],
                            start=(ec == 0),
                            stop=(ec == EC - 1),
                        )
                    th = tanh_pool.tile([P, 512], BF16, tag="tanh")
                    nc.scalar.activation(
                        th[:, :w], mm_ps[:, :w], Tanh,
                        bias=projs_sb[:, hc, b : b + 1],
                    )
                    ths.append(th)
                    if hc == 0:
                        emit_v()
                state["v"] = (b, pi, off, w, ths)
            emit_v()
            emit_reduce()

    _dedup_ldweights(nc)
    _split_multiwaits(nc)
    return nc


def _prep_inputs(s, encoder_outputs, attn_w, v_w):
    s = np.asarray(s, dtype=np.float32)
    enc = np.asarray(encoder_outputs, dtype=np.float32)
    attn_w = np.asarray(attn_w, dtype=np.float32)
    v_w = np.asarray(v_w, dtype=np.float32)

    w_et = np.ascontiguousarray(attn_w[:, D:].T).astype(NP_BF16)  # [E, H]
    w_st = np.ascontiguousarray(attn_w[:, :D].T).astype(NP_BF16)  # [D, H]
    v_rep = np.ascontiguousarray(
        np.repeat(v_w.T.astype(NP_BF16), 32, axis=1)
    )  # [H, 32]
    red_t = np.full((P, 1), 1.0 / 32.0, dtype=NP_BF16)

    in_maps = []
    for c in range(N_CORES):
        lo, hi = c * BPC, (c + 1) * BPC
        # [BPC, E, S] bf16: pre-cast on host so the device DMA reads half
        # the HBM bytes (the f32->bf16 cast-on-load path reads f32)
        enc_t = np.ascontiguousarray(
            enc[lo:hi].astype(NP_BF16).transpose(0, 2, 1)
        )
        s_t = np.ascontiguousarray(s[lo:hi].T).astype(NP_BF16)  # [D, BPC]
        in_maps.append(
            {
                "enc_t": enc_t,
                "w_et": w_et,
                "w_st": w_st,
                "s_t": s_t,
                "v_rep": v_rep,
                "red_t": red_t,
            }
        )
    return in_maps


def _run(s, encoder_outputs, attn_w, v_w, trace=False):
    if "nc" not in _cache:
        _cache["nc"] = _build_bass()
    nc = _cache["nc"]
    in_maps = _prep_inputs(s, encoder_outputs, attn_w, v_w)
    res = run_bass_kernel_spmd(nc, in_maps, list(range(N_CORES)), trace=trace)
    out = np.concatenate([res.results[c]["out"] for c in range(N_CORES)], axis=0)
    return out.astype(np.float32), res


def kernel(s, encoder_outputs, attn_w, v_w):
    out, _ = _run(s, encoder_outputs, attn_w, v_w, trace=False)
    return out


# revision 15
# speedup vs baseline: 1.1083x; 1.1083x over previous
"""Bahdanau-style attention scores kernel for Trainium2 (8 NeuronCores).

Reference computation (B=32, S=2048, ENC_H=512, DEC_H=1024):
    W_s = attn_w[:, :1024]; W_e = attn_w[:, 1024:]
    proj_s = s @ W_s.T                      # [B, 1024]
    proj_e = enc @ W_e.T                    # [B, S, 1024]
    scores = tanh(proj_s[:, None] + proj_e) @ v_w.T   # [B, S]
    out = softmax(scores, axis=1)

Strategy: data-parallel over batch (4 batches per core). Everything is
core-local, including the softmax, so there are no collectives.

On-device layout: all matmuls keep the hidden dim h on PSUM partitions:
    projT[h, s] = sum_e W_eT[e, h] * encT[e, s]
so the per-batch proj_s bias is a per-partition scalar (fused into the
ACT tanh) and the v-dot runs as 2 rounds of 4 concurrent col-tiled
matmuls (M=32 with v replicated across 32 columns so every PSUM
partition is written), followed by a 1/32-weighted reduce matmul.

Prologue engineering: a short stream of dummy matmuls keeps the PE HAM
activity monitor busy from ~7us so real matmuls run at 2.4 GHz instead
of the cold 1.2 GHz; weights arrive in h-quarter chunks (hc-major) so
the first matmul/tanh groups gate on ~512KB instead of 2MB; the first
batch's encoder stream lands in small leading pieces. proj_s matmuls
are interleaved with the first piece's main groups so neither blocks
the other. The host passes encoder_outputs pre-transposed to [b, E, S]
(pure layout change, f32); f32 -> bf16 conversion of the enc stream
happens inside the SWDGE DMA (cast-on-load). The small replicated
weights are pre-cast to bf16 on the host and loaded via HWDGE.
"""

import numpy as np
import ml_dtypes

import concourse.bass as bass
import concourse.tile as tile
from concourse import mybir
from concourse.bass_utils import run_bass_kernel_spmd

N_CORES = 8
B, S = 32, 2048
E = 1024  # 2*ENC_H, contraction dim of the big matmul
H = 1024  # DEC_H, hidden dim of tanh
D = 1024  # DEC_H, contraction dim of proj_s
BPC = B // N_CORES  # batches per core
P = 128
EC, HC, DC = E // P, H // P, D // P

# s-piece schedule: batch 0 starts small so the first matmul group gates
# on ~0.5MB of enc DMA, later batches use full 512 pieces.
PIECES_B0 = [256, 256, 512, 512, 512]
PIECES = [512] * 4
N_DUMMY = 250  # HAM warm-up matmuls (N=8 each, ~30ns apiece)

F32 = mybir.dt.float32
BF16 = mybir.dt.bfloat16
NP_BF16 = ml_dtypes.bfloat16

_cache = {}


def _split_multiwaits(nc):
    """Walrus in this toolchain rejects instructions carrying more than one
    semaphore wait ("Too many sync wait commands"). Engine queues dispatch in
    order, so moving the extra waits onto same-engine NoOps just before the
    instruction is semantically identical."""
    for fn in nc.m.functions:
        for blk in fn.blocks:
            out = []
            for inst in blk.instructions:
                si = inst.sync_info
                waits = list(si.on_wait) if si is not None and si.on_wait else []
                if len(waits) > 1:
                    for i, w in enumerate(waits[:-1]):
                        out.append(
                            mybir.InstNoOp(
                                name=f"{inst.name}-w{i}",
                                engine=inst.engine,
                                sync_info=mybir.SyncInfo(on_wait=[w], on_update=[]),
                                bass_nofuse=True,
                            )
                        )
                    si.on_wait = [waits[-1]]
                    inst.sync_info = si
                out.append(inst)
            try:
                blk.instructions = out
            except Exception:
                blk.set_instructions(out)


def _dedup_ldweights(nc):
    """Tile lowers every matmul to an Ldweights/Matmult pair. When consecutive
    matmuls use the same stationary weights (the dummy warm-up stream), the
    second Ldweights reloads identical array state — drop it and carry its
    waits over to the next PE instruction (split later by _split_multiwaits)."""
    ndrop = 0
    for fn in nc.m.functions:
        for blk in fn.blocks:
            out = []
            loaded = None
            pending_waits = []
            for inst in blk.instructions:
                if getattr(inst, "engine", None) != mybir.EngineType.PE:
                    out.append(inst)
                    continue
                if pending_waits:
                    si = inst.sync_info or mybir.SyncInfo(on_wait=[], on_update=[])
                    si.on_wait = list(si.on_wait) + pending_waits
                    inst.sync_info = si
                    pending_waits = []
                if isinstance(inst, mybir.InstLdweights):
                    ap = inst.ins[0]
                    key = (
                        ap.memref,
                        ap.offset,
                        str(ap.ap),
                        str(ap.dtype),
                        str(getattr(inst, "tile_position", None)),
                    )
                    if key == loaded:
                        si = inst.sync_info
                        if si is not None and si.on_wait:
                            pending_waits = list(si.on_wait)
                        if si is not None and si.on_update:
                            # keep the instruction if someone depends on it
                            out.append(inst)
                            continue
                        ndrop += 1
                        continue
                    loaded = key
                elif isinstance(inst, mybir.InstMatmult):
                    pass  # matmuls stream against loaded weights
                else:
                    loaded = None  # unknown PE instruction: be conservative
                out.append(inst)
            assert not pending_waits
            try:
                blk.instructions = out
            except Exception:
                blk.set_instructions(out)
    return ndrop


def _pieces(b):
    sched = PIECES_B0 if b == 0 else PIECES
    off = 0
    out = []
    for w in sched:
        out.append((off, w))
        off += w
    return out


def _build_bass():
    nc = bass.Bass()
    # All weight tensors arrive pre-swizzled into partition-major slabs so
    # every DMA is 128 descriptors of >=4KB (descriptor count, not bytes,
    # is what throttles the DGE rings).
    enc_t = nc.dram_tensor("enc_t", [BPC, E, S], BF16, kind="ExternalInput")
    w_q = nc.dram_tensor("w_q", [4, P, EC * 256], BF16, kind="ExternalInput")
    ws_q = nc.dram_tensor("ws_q", [4, P, DC * 256], BF16, kind="ExternalInput")
    s_p = nc.dram_tensor("s_p", [P, DC * BPC], BF16, kind="ExternalInput")
    vr_p = nc.dram_tensor("vr_p", [P, HC * 32 + 1], BF16, kind="ExternalInput")
    out = nc.dram_tensor("out", [BPC, S], F32, kind="ExternalOutput")

    Tanh = mybir.ActivationFunctionType.Tanh
    Exp = mybir.ActivationFunctionType.Exp

    with tile.TileContext(nc) as tc:
        with (
            tc.tile_pool(name="consts", bufs=1) as consts,
            tc.tile_pool(name="enc", bufs=3) as enc_pool,
            tc.tile_pool(name="tanh", bufs=10) as tanh_pool,
            tc.tile_pool(name="scc", bufs=2) as scc_pool,
            tc.tile_pool(name="rows", bufs=2) as row_pool,
            tc.tile_pool(name="mmps", bufs=4, space="PSUM") as mm_psum,
            tc.tile_pool(name="scps", bufs=2, space="PSUM") as sc_psum,
            tc.tile_pool(name="psps", bufs=2, space="PSUM") as ps_psum,
        ):
            # HAM warm-up: PE busy from the end of the framework preamble so
            # the clock gate opens (1.2 -> 2.4 GHz) before real work arrives.
            # The dummy tile is memset (not DMA'd) so nothing gates it.
            dummy = consts.tile([P, 8], BF16)
            nc.vector.memset(dummy, 0.0)
            dps = ps_psum.tile([1, 8], F32, tag="psps")
            for _ in range(N_DUMMY):
                nc.tensor.matmul(dps, dummy[:, 0:1], dummy, start=True, stop=True)

            # Weights in h-quarter chunks, hc-major, so the first tanh/main
            # groups gate on 512KB not 2MB. Two HWDGE rings: sync carries
            # W_e, scalar carries W_s plus the tiny tensors (first).
            w_sb = consts.tile([P, 4, EC, 256], BF16)
            ws_sb = consts.tile([P, 4, DC, 256], BF16)
            s_sb = consts.tile([P, DC, BPC], BF16)
            nc.scalar.dma_start(
                out=s_sb[:], in_=s_p[:].rearrange("p (dc b) -> p dc b", dc=DC)
            )
            vr_sb = consts.tile([P, HC * 32 + 1], BF16)
            nc.scalar.dma_start(out=vr_sb[:], in_=vr_p[:])
            red_sb = vr_sb[:, HC * 32 : HC * 32 + 1]
            for q in range(4):
                nc.sync.dma_start(
                    out=w_sb[:, q],
                    in_=w_q[q].rearrange("p (ec c) -> p ec c", ec=EC),
                )
                nc.scalar.dma_start(
                    out=ws_sb[:, q],
                    in_=ws_q[q].rearrange("p (dc c) -> p dc c", dc=DC),
                )

            def w_tile(sb, ec, hc):
                return sb[:, hc // 2, ec, (hc % 2) * P : (hc % 2 + 1) * P]

            projs_sb = consts.tile([P, HC, BPC], F32)

            def projs_chunk(hc):
                # projsT[h, b] = sum_d W_sT[d, h] * sT[d, b] for one h-chunk
                pp = ps_psum.tile([P, BPC], F32, tag="psps")
                for dc in range(DC):
                    nc.tensor.matmul(
                        pp,
                        w_tile(ws_sb, dc, hc),
                        s_sb[:, dc, :],
                        start=(dc == 0),
                        stop=(dc == DC - 1),
                    )
                nc.vector.tensor_copy(projs_sb[:, hc, :], pp)

            # Flat pipeline over all (batch, piece) jobs. Per piece p the PE
            # stream is: [mm groups hc=0..7 of p] ... with p's v-rounds and
            # the reduce-mm of p-1 emitted after the FIRST mm group of p+1,
            # so the v-rounds never wait on p's last tanh and the reduce-mm
            # never waits on the DVE psum->sbuf copy.
            enc_tiles = {}
            for b in range(BPC):
                encT = enc_pool.tile([P, EC, S], BF16, tag="enc", name=f"encT{b}")
                enc_view = enc_t[b].rearrange("(ec p) s -> p ec s", p=P)
                if b == 0:
                    for off, w in _pieces(b):
                        sl = slice(off, off + w)
                        nc.gpsimd.dma_start(
                            out=encT[:, :, sl], in_=enc_view[:, :, sl]
                        )
                else:
                    for half in range(2):
                        sl = slice(half * 1024, (half + 1) * 1024)
                        nc.gpsimd.dma_start(
                            out=encT[:, :, sl], in_=enc_view[:, :, sl]
                        )
                enc_tiles[b] = encT

            rows = {}  # b -> (exp_row, sums)
            jobs = []
            for b in range(BPC):
                for pi, (off, w) in enumerate(_pieces(b)):
                    jobs.append((b, pi, off, w))

            state = {"v": None, "red": None}

            def emit_reduce():
                # reduce-mm + exp of the piece whose psum->sbuf copy is done
                if state["red"] is None:
                    return
                b, pi, off, w, scc = state["red"]
                state["red"] = None
                exp_row, sums = rows[b]
                sc2 = ps_psum.tile([1, 512], F32, tag="psps")
                nc.tensor.matmul(
                    sc2[:, :w], red_sb, scc[:, :w], start=True, stop=True
                )
                nc.scalar.activation(
                    exp_row[:, off : off + w],
                    sc2[:, :w],
                    Exp,
                    accum_out=sums[:, pi : pi + 1],
                )
                if pi == len(_pieces(b)) - 1:
                    npc = len(_pieces(b))
                    tot = row_pool.tile([1, 1], F32, tag="tot")
                    nc.vector.reduce_sum(
                        tot, sums[:, :npc], axis=mybir.AxisListType.X
                    )
                    rtot = row_pool.tile([1, 1], F32, tag="rtot")
                    nc.vector.reciprocal(rtot, tot)
                    out_row = row_pool.tile([1, S], F32, tag="out_row")
                    nc.vector.tensor_scalar_mul(out_row, exp_row, rtot)
                    nc.sync.dma_start(out=out[b : b + 1, :], in_=out_row[:])

            def emit_v():
                # v-dot of the piece whose tanh tiles are all complete:
                # 2 rounds x 4 concurrent col-tiled matmuls. v is replicated
                # over 32 columns so all 128 PSUM partitions hold valid
                # partials (no garbage for the reduce matmul).
                if state["v"] is None:
                    return
                b, pi, off, w, ths = state["v"]
                state["v"] = None
                sc_ps = sc_psum.tile([P, 512], F32, tag="scps")
                for r in range(2):
                    for j in range(4):
                        hc = r * 4 + j
                        nc.tensor.matmul(
                            sc_ps[32 * j : 32 * (j + 1), :w],
                            vr_sb[:, hc * 32 : (hc + 1) * 32],
                            ths[hc][:, :w],
                            start=(r == 0),
                            stop=(r == 1),
                            tile_position=(0, 32 * j),
                        )
                emit_reduce()
                scc = scc_pool.tile([P, 512], BF16, tag="scc")
                nc.vector.tensor_copy(scc[:, :w], sc_ps[:, :w])
                state["red"] = (b, pi, off, w, scc)

            for b, pi, off, w in jobs:
                if pi == 0:
                    exp_row = row_pool.tile([1, S], F32, tag="exp_row")
                    sums = row_pool.tile(
                        [1, len(PIECES_B0)], F32, tag="sums"
                    )
                    rows[b] = (exp_row, sums)
                encT = enc_tiles[b]
                sl = slice(off, off + w)
                ths = []
                for hc in range(HC):
                    if b == 0 and pi == 0:
                        projs_chunk(hc)
                    mm_ps = mm_psum.tile([P, 512], F32, tag="mmps")
                    for ec in range(EC):
                        nc.tensor.matmul(
                            mm_ps[:, :w],
                            w_tile(w_sb, ec, hc),
                            encT[:, ec, sl],
                            start=(ec == 0),
                            stop=(ec == EC - 1),
                        )
                    th = tanh_pool.tile([P, 512], BF16, tag="tanh")
                    nc.scalar.activation(
                        th[:, :w], mm_ps[:, :w], Tanh,
                        bias=projs_sb[:, hc, b : b + 1],
                    )
                    ths.append(th)
                    if hc == 0:
                        emit_v()
                state["v"] = (b, pi, off, w, ths)
            emit_v()
            emit_reduce()

    _dedup_ldweights(nc)
    _split_multiwaits(nc)
    return nc


def _prep_inputs(s, encoder_outputs, attn_w, v_w):
    s = np.asarray(s, dtype=np.float32)
    enc = np.asarray(encoder_outputs, dtype=np.float32)
    attn_w = np.asarray(attn_w, dtype=np.float32)
    v_w = np.asarray(v_w, dtype=np.float32)

    def quarter_slabs(w_t):
        # [X, H] -> [4, P, XC*256] partition-major h-quarter slabs
        xc = w_t.shape[0] // P
        tmp = w_t.astype(NP_BF16).reshape(xc, P, 4, 256)
        return np.ascontiguousarray(
            tmp.transpose(2, 1, 0, 3).reshape(4, P, xc * 256)
        )

    w_q = quarter_slabs(attn_w[:, D:].T)  # from [E, H]
    ws_q = quarter_slabs(attn_w[:, :D].T)  # from [D, H]
    # [P, HC*32+1]: v replicated over 32 cols per h-chunk, then the 1/32
    # reduction weight in the last column
    v_t = v_w.reshape(HC, P).T.astype(NP_BF16)  # [P, HC]
    vr_p = np.concatenate(
        [
            np.repeat(v_t, 32, axis=1),
            np.full((P, 1), 1.0 / 32.0, dtype=NP_BF16),
        ],
        axis=1,
    )
    vr_p = np.ascontiguousarray(vr_p)

    in_maps = []
    for c in range(N_CORES):
        lo, hi = c * BPC, (c + 1) * BPC
        # [BPC, E, S] bf16: pre-cast on host so the device DMA reads half
        # the HBM bytes (the f32->bf16 cast-on-load path reads f32)
        enc_t = np.ascontiguousarray(
            enc[lo:hi].astype(NP_BF16).transpose(0, 2, 1)
        )
        # [P, DC*BPC] partition-major packing of s^T
        s_p = np.ascontiguousarray(
            s[lo:hi].T.astype(NP_BF16).reshape(DC, P, BPC)
            .transpose(1, 0, 2).reshape(P, DC * BPC)
        )
        in_maps.append(
            {"enc_t": enc_t, "w_q": w_q, "ws_q": ws_q, "s_p": s_p, "vr_p": vr_p}
        )
    return in_maps


def _run(s, encoder_outputs, attn_w, v_w, trace=False):
    if "nc" not in _cache:
        _cache["nc"] = _build_bass()
    nc = _cache["nc"]
    in_maps = _prep_inputs(s, encoder_outputs, attn_w, v_w)
    res = run_bass_kernel_spmd(nc, in_maps, list(range(N_CORES)), trace=trace)
    out = np.concatenate([res.results[c]["out"] for c in range(N_CORES)], axis=0)
    return out.astype(np.float32), res


def kernel(s, encoder_outputs, attn_w, v_w):
    out, _ = _run(s, encoder_outputs, attn_w, v_w, trace=False)
    return out
